# revision 1
# baseline (speedup 1.0000x reference)
"""Trainium2 Bass kernel for nn_BinaryConv2d (B=16, C=64, H=W=256, 3x3, pad 1).

Forward semantics (STE forward values):
  act = sign(x * rd_k + rd_b)                  in {-1, 0, +1}
  bw  = scaling[co] * sign(conv_w)             scaling = mean |conv_w| per out-ch
  y   = conv2d(act, bw, pad=1)
  y   = prelu(y + pr_bias0) + pr_bias1 + x     prelu slope per channel

Strategy: data-parallel over batch, 2 images per core (8 cores).  The two
images' 64 channels are stacked on the 128 SBUF partitions.  Activations are
binarized to bf16 +-1 on the Scalar engine; the 3x3 conv is 9 accumulating
PSUM matmuls with block-diagonal +-1 bf16 weights (exact integer arithmetic
in fp32 PSUM).  Per-channel scaling / PReLU / biases / residual are fused
post-ops on ScalarE / GpSimd / VectorE.
"""

import sys

if "/opt/trn_rl_repo" not in sys.path:
    sys.path.insert(0, "/opt/trn_rl_repo")

from contextlib import ExitStack

import ml_dtypes
import numpy as np

import concourse.bacc as bacc
import concourse.bass as bass
import concourse.tile as tile
from concourse import mybir
from concourse.bass_utils import run_bass_kernel_spmd

B, C, H, W = 16, 64, 256, 256
NCORES = 8
HS = 32                      # output rows per strip
NSTRIPS = H // HS
P = 128                      # partitions = 2 images x 64 channels

F32 = mybir.dt.float32
BF16 = mybir.dt.bfloat16
FP8 = mybir.dt.float8e4
AF = mybir.ActivationFunctionType
ALU = mybir.AluOpType

# 'bf16': 9 matmuls (K=128) per 2-row tile on bf16 +-1 operands.
# 'fp8dr': activations/weights in fp8e4; the kh=0/1 pairs are packed into
#   DoubleRow matmuls (2 MACs/cell/cycle), cutting PE streaming ~29%.
#   All values are exactly representable (+-1, 0), so precision is identical.
CONV_MODE = "fp8dr"
APITCH = 272                 # act row pitch (bytes %16 for DoubleRow AP steps)

# Param table columns (per-partition f32 scalars)
PK, PB, PS, PB0, PCM, PB1, PSL = 0, 1, 2, 3, 4, 5, 6

# The ACT-engine Lrelu activation computes something other than
# prelu(x, alpha) on TRN2 hardware (measured absmax 0.1 vs reference), so the
# PReLU is done on the Vector engine instead.
USE_LRELU = False

# prelu(u) == max(u, slope*u) when 0 <= slope <= 1 (checked at runtime in
# kernel()); one fused scalar_tensor_tensor op instead of tensor_scalar +
# tensor_tensor.  Set False for the general min/mult path.
PRELU_MAX_TRICK = True

SIGN_CHUNK = 9               # rows of sign-activation per ACT instruction
TILE_ROWS = 2                # output rows per PSUM tile (2 or 4)
STRIP_HS = [32] * 8          # strip heights (sum == H)


def _emit(tc, nc, x_d, w_d, p_d, y_d):
    x3 = x_d.rearrange("p (h w) -> p h w", w=W)
    y3 = y_d.rearrange("p (h w) -> p h w", w=W)
    fp8dr = CONV_MODE == "fp8dr"
    adt = FP8 if fp8dr else BF16
    apitch = APITCH if fp8dr else W + 2

    with ExitStack() as ctx:
        consts = ctx.enter_context(tc.tile_pool(name="consts", bufs=1))
        xpool = ctx.enter_context(tc.tile_pool(name="xpool", bufs=2))
        apool = ctx.enter_context(tc.tile_pool(name="apool", bufs=2))
        ypool = ctx.enter_context(tc.tile_pool(name="ypool", bufs=2))
        nps = 16 // TILE_ROWS    # PSUM tiles cycling through all 8 banks
        tpool = ctx.enter_context(tc.tile_pool(name="tpool", bufs=nps))
        pspool = ctx.enter_context(tc.tile_pool(name="pspool", bufs=nps,
                                                space="PSUM"))

        # params first on the load ring (sign needs them); weights on the
        # store ring, which is idle at kernel start -- the first x chunk
        # then starts immediately behind pt instead of behind the weights
        pt = consts.tile([P, 8], F32)
        nc.sync.dma_start(out=pt, in_=p_d)
        if fp8dr:
            # [kw, delta(kh 0/1), m] DoubleRow weights + [kw, m] kh=2 weights
            wdr = consts.tile([P, 3, 2, 128], FP8)
            nc.scalar.dma_start(out=wdr, in_=w_d[:, :768].rearrange(
                "p (k d m) -> p k d m", k=3, d=2))
            wn = consts.tile([P, 3, 128], FP8)
            nc.scalar.dma_start(out=wn, in_=w_d[:, 768:].rearrange(
                "p (k m) -> p k m", k=3))
        else:
            wt = consts.tile([P, 9, 128], BF16)
            nc.scalar.dma_start(out=wt,
                                in_=w_d.rearrange("p (j m) -> p j m", j=9))

        # Uneven strips: a tiny final strip shortens the kernel tail (the
        # last strip's post-op/store chain cannot overlap further matmuls).
        H0S = [sum(STRIP_HS[:i]) for i in range(len(STRIP_HS))]
        NST = len(STRIP_HS)
        HSMAX = max(STRIP_HS)

        def strip_rows(s):
            h0 = H0S[s]
            row_lo = max(h0 - 1, 0)
            row_hi = min(h0 + STRIP_HS[s] + 1, H)
            return h0, row_lo, row_hi, row_lo - (h0 - 1)

        def load_strip(s):
            """DMA the x strip (rows h0-1 .. h0+hs; tile row a <-> global
            h0-1+a) and memset the act padding."""
            h0, row_lo, row_hi, r0 = strip_rows(s)
            nr = row_hi - row_lo
            xs = xpool.tile([P, HSMAX + 2, W], F32, name="xs")
            nld = 4 if s == 0 else 2     # strip 0 in quarters: faster start
            bounds = [row_lo + (nr * i) // nld for i in range(nld + 1)]
            for a, b in zip(bounds, bounds[1:]):
                if b > a:
                    nc.sync.dma_start(out=xs[:, a - (h0 - 1):b - (h0 - 1), :],
                                      in_=x3[:, a:b, :])
            act = apool.tile([P, HSMAX + 2, apitch], adt, name="act")
            nrows = STRIP_HS[s] + 2
            nc.gpsimd.memset(act[:, :nrows, 0:1], 0.0)
            nc.gpsimd.memset(act[:, :nrows, W + 1:W + 2], 0.0)
            if s == 0:
                nc.gpsimd.memset(act[:, 0:1, :], 0.0)
            if s == NST - 1:
                nc.gpsimd.memset(act[:, nrows - 1:nrows, :], 0.0)
            return xs, act

        def sign_strip(s, xs, act, chunks, skip=0):
            """Binarize x into the zero-padded act tile, in row chunks (the
            first small so dependent matmuls unblock quickly)."""
            _, row_lo, row_hi, r0 = strip_rows(s)
            c0 = r0 + skip
            for sz in chunks:
                c1 = min(c0 + sz, r0 + (row_hi - row_lo))
                if c1 <= c0:
                    break
                nc.scalar.activation(
                    act[:, c0:c1, 1:W + 1], xs[:, c0:c1, :], AF.Sign,
                    bias=pt[:, PB:PB + 1], scale=pt[:, PK:PK + 1],
                )
                c0 = c1

        FIRST_CHUNKS = (5,) * 7 + (4,)   # strip 0: progressive chunks
        NEXT_CHUNKS = (5,) + (SIGN_CHUNK,) * 4

        # Pipelining the whole next-strip sign block ahead of this strip's
        # tail post-ops measured slower (ACT head-of-line effects); only the
        # first small chunk is hoisted so the next strip's first matmuls
        # don't wait for the full ACT drain.
        PIPELINED_SIGN = False
        SPLIT_FIRST_CHUNK = True
        cur = load_strip(0)
        sign_strip(0, *cur, FIRST_CHUNKS)
        nxt = None
        for s in range(NST):
            h0 = H0S[s]
            HS_S = STRIP_HS[s]
            MT = HS_S // TILE_ROWS
            xs, act = cur
            ys = ypool.tile([P, HSMAX, W], F32, name="ys")
            for mt in range(MT):
                if mt == min(1, MT - 1) and s + 1 < NST and (
                        PIPELINED_SIGN or SPLIT_FIRST_CHUNK):
                    nxt = load_strip(s + 1)   # loads overlap this strip
                if mt == max(MT - 2, 0) and s + 1 < NST:
                    # data definitely landed; ACT binarizes it while the PE
                    # finishes this strip
                    if PIPELINED_SIGN:
                        sign_strip(s + 1, *nxt, NEXT_CHUNKS)
                    elif SPLIT_FIRST_CHUNK:
                        sign_strip(s + 1, *nxt, NEXT_CHUNKS[:1])
                ps = pspool.tile([P, TILE_ROWS, W], F32, name="ps")
                for half in range(TILE_ROWS // 2):
                    r = TILE_ROWS * mt + 2 * half  # first output row of pair
                    po = ps[:, 2 * half:2 * half + 2, :]
                    if fp8dr:
                        for kw in range(3):
                            for i in range(2):
                                # kh in {0,1} via DoubleRow: contraction over
                                # (partition, delta), act row (r+i)+delta
                                nc.tensor.matmul(
                                    po[:, i, :],
                                    lhsT=wdr[:, kw, :, :],
                                    rhs=act[:, r + i:r + i + 2, kw:kw + W],
                                    start=(kw == 0 and i == 0),
                                    stop=False,
                                    perf_mode=mybir.MatmulPerfMode.DoubleRow,
                                )
                        for kw in range(3):
                            # kh=2 plain matmul over both output rows
                            nc.tensor.matmul(
                                po,
                                lhsT=wn[:, kw, :],
                                rhs=act[:, r + 2:r + 4, kw:kw + W],
                                start=False,
                                stop=(kw == 2),
                            )
                    else:
                        for j in range(9):
                            kh, kw = divmod(j, 3)
                            nc.tensor.matmul(
                                po,
                                lhsT=wt[:, j, :],
                                rhs=act[:, r + kh:r + kh + 2, kw:kw + W],
                                start=(j == 0),
                                stop=(j == 8),
                            )
                r = TILE_ROWS * mt
                u = ys[:, r:r + TILE_ROWS, :]
                xres = xs[:, r + 1:r + 1 + TILE_ROWS, :]
                if PRELU_MAX_TRICK:
                    # v = ps*scaling + b0 (per 2-row psum tile on ACT); the
                    # prelu/residual stt ops run at 4-row granularity so the
                    # DVE pays its inter-op SBUF bubble half as often
                    if mt % 2 == 0:
                        v4 = tpool.tile([P, 2 * TILE_ROWS, W], F32, name="v")
                    half = (mt % 2) * TILE_ROWS
                    nc.scalar.activation(
                        v4[:, half:half + TILE_ROWS, :], ps, AF.Identity,
                        bias=pt[:, PB0:PB0 + 1], scale=pt[:, PS:PS + 1],
                    )
                    if mt % 2 == 1:
                        r4 = TILE_ROWS * (mt - 1)
                        u4 = ys[:, r4:r4 + 2 * TILE_ROWS, :]
                        x4 = xs[:, r4 + 1:r4 + 1 + 2 * TILE_ROWS, :]
                        nc.vector.scalar_tensor_tensor(
                            u4, v4, pt[:, PSL:PSL + 1], v4, ALU.mult, ALU.max
                        )
                        nc.vector.scalar_tensor_tensor(
                            u4, x4, pt[:, PB1:PB1 + 1], u4, ALU.add, ALU.add
                        )
                else:
                    # u = ps*scaling + b0 ; u += (slope-1)*min(u, 0); u += x+b1
                    nc.scalar.activation(
                        u, ps, AF.Identity,
                        bias=pt[:, PB0:PB0 + 1], scale=pt[:, PS:PS + 1],
                    )
                    m = tpool.tile([P, TILE_ROWS, W], F32, name="m")
                    nc.vector.tensor_scalar(
                        m, u, 0.0, pt[:, PCM:PCM + 1], ALU.min, ALU.mult
                    )
                    nc.vector.tensor_tensor(u, u, m, ALU.add)
                    nc.vector.scalar_tensor_tensor(
                        u, xres, pt[:, PB1:PB1 + 1], u, ALU.add, ALU.add
                    )
            # stores on the ACT HWDGE ring (separate queue from loads)
            nq = 2 if HS_S > 8 else 1
            for q in range(nq):
                r = q * (HS_S // nq)
                r1 = (q + 1) * (HS_S // nq)
                nc.scalar.dma_start(out=y3[:, h0 + r:h0 + r1, :],
                                    in_=ys[:, r:r1, :])
            if s + 1 < NST and not PIPELINED_SIGN:
                if SPLIT_FIRST_CHUNK:
                    sign_strip(s + 1, *nxt, NEXT_CHUNKS[1:],
                               skip=NEXT_CHUNKS[0])
                else:
                    nxt = load_strip(s + 1)
                    sign_strip(s + 1, *nxt, NEXT_CHUNKS)
            cur = nxt


def build_nc():
    nc = bacc.Bacc("TRN2", target_bir_lowering=False, debug=False,
                   num_devices=NCORES)
    wdt = FP8 if CONV_MODE == "fp8dr" else BF16
    x_d = nc.dram_tensor("xin", [P, H * W], F32, kind="ExternalInput").ap()
    w_d = nc.dram_tensor("wp", [P, 9 * 128], wdt, kind="ExternalInput").ap()
    p_d = nc.dram_tensor("pp", [P, 8], F32, kind="ExternalInput").ap()
    y_d = nc.dram_tensor("yout", [P, H * W], F32, kind="ExternalOutput").ap()
    with tile.TileContext(nc) as tc:
        _emit(tc, nc, x_d, w_d, p_d, y_d)
    nc.compile()
    return nc


_NC_CACHE = {}


def _get_nc():
    key = (USE_LRELU, PRELU_MAX_TRICK, CONV_MODE)
    if key not in _NC_CACHE:
        _NC_CACHE[key] = build_nc()
    return _NC_CACHE[key]


def make_inputs(x, rd_k, rd_b, beta, conv_w, pr_bias0, prelu_w, pr_bias1):
    """Host-side prep: per-channel param table, packed sign weights, shards."""
    k = np.asarray(rd_k, np.float32).reshape(C)
    b = np.asarray(rd_b, np.float32).reshape(C)
    s = np.mean(np.abs(np.asarray(conv_w, np.float32)), axis=(1, 2, 3))
    b0 = np.asarray(pr_bias0, np.float32).reshape(C)
    slope = np.asarray(prelu_w, np.float32).reshape(C)
    b1 = np.asarray(pr_bias1, np.float32).reshape(C)
    cm = slope - 1.0
    cols = np.stack([k, b, s, b0, cm, b1, slope, np.zeros(C, np.float32)], axis=1)
    pp = np.concatenate([cols, cols], axis=0).astype(np.float32)  # [128, 8]

    sw = np.sign(np.asarray(conv_w, np.float32)).astype(np.float32)  # [co,ci,kh,kw]

    def blockdiag(kh, kw):
        S = sw[:, :, kh, kw].T  # [ci, co]
        out = np.zeros((P, P), np.float32)
        out[0:C, 0:C] = S
        out[C:P, C:P] = S
        return out

    if CONV_MODE == "fp8dr":
        wp = np.zeros((P, 9, 128), np.float32)
        for kw in range(3):            # [kw, delta, m] DoubleRow pairs
            for d in range(2):
                wp[:, kw * 2 + d, :] = blockdiag(d, kw)
        for kw in range(3):            # [kw, m] kh=2
            wp[:, 6 + kw, :] = blockdiag(2, kw)
        wdt = mybir.dt.np(FP8)
    else:
        wp = np.zeros((P, 9, 128), np.float32)
        for j in range(9):
            kh, kw = divmod(j, 3)
            wp[:, j, :] = blockdiag(kh, kw)
        wdt = ml_dtypes.bfloat16
    wp = np.ascontiguousarray(wp.reshape(P, 9 * 128)).astype(wdt)

    x = np.asarray(x, np.float32)
    in_maps = []
    for c in range(NCORES):
        xc = np.ascontiguousarray(x[2 * c:2 * c + 2]).reshape(P, H * W)
        in_maps.append({"xin": xc, "wp": wp, "pp": pp})
    return in_maps


def kernel(x, rd_k, rd_b, beta, conv_w, pr_bias0, prelu_w, pr_bias1):
    global PRELU_MAX_TRICK
    slope = np.asarray(prelu_w, np.float32).reshape(C)
    if not np.all((slope >= 0.0) & (slope <= 1.0)):
        PRELU_MAX_TRICK = False   # max-identity only valid for slope in [0,1]
    in_maps = make_inputs(x, rd_k, rd_b, beta, conv_w, pr_bias0, prelu_w,
                          pr_bias1)
    nc = _get_nc()
    res = run_bass_kernel_spmd(nc, in_maps, core_ids=list(range(NCORES)))
    y = np.empty((B, C, H, W), np.float32)
    for c in range(NCORES):
        y[2 * c:2 * c + 2] = res.results[c]["yout"].reshape(2, C, H, W)
    return y



# revision 4
# speedup vs baseline: 1.1746x; 1.1746x over previous
"""Trainium2 Bass kernel for nn_BinaryConv2d (B=16, C=64, H=W=256, 3x3, pad 1).

Forward semantics (STE forward values):
  act = sign(x * rd_k + rd_b)                  in {-1, 0, +1}
  bw  = scaling[co] * sign(conv_w)             scaling = mean |conv_w| per out-ch
  y   = conv2d(act, bw, pad=1)
  y   = prelu(y + pr_bias0) + pr_bias1 + x     prelu slope per channel

Strategy: data-parallel over batch, 2 images per core (8 cores).  The two
images' 64 channels are stacked on the 128 SBUF partitions.  x is shipped to
the device in bf16 (halves HBM reads); y is produced in bf16 and upcast on
the host (halves HBM writes).  Activations are binarized to fp8 +-1 on the
Scalar engine.  The 3x3 conv runs as fp8 DoubleRow matmuls with
block-diagonal +-1 weights (exact integer arithmetic in fp32 PSUM):

  - taps (kh=0,kw)+(kh=1,kw) pair along the act row stride (3 DR matmuls
    per output row),
  - taps (kh=2,kw=0)+(kh=2,kw=1) pair across a column-shifted copy of the
    act plane (delta stride = plane pitch, 16B-aligned as DR requires); the
    shifted plane is produced by a cheap SBUF->SBUF DMA,
  - tap (kh=2,kw=2) is a plain matmul over a 2-row pair.

That is 5 PE streaming cycles per output column (vs 6 for the kh-pair-only
scheme); the 9-tap/DoubleRow parity floor is 4.5.

PSUM is organized as two 8-row tiles (4 banks each) so the per-channel
scale/bias PSUM->SBUF drain is one ACT instruction per 8 rows, and the
PReLU + residual run as two bf16 scalar_tensor_tensor ops per 8 rows on the
Vector engine (bf16 = 2x DVE rate).
"""

import sys

if "/opt/trn_rl_repo" not in sys.path:
    sys.path.insert(0, "/opt/trn_rl_repo")

from contextlib import ExitStack

import ml_dtypes
import numpy as np

import concourse.bacc as bacc
import concourse.bass as bass
import concourse.tile as tile
from concourse import mybir
from concourse.bass_utils import run_bass_kernel_spmd

B, C, H, W = 16, 64, 256, 256
NCORES = 8
HS = 32                      # output rows per strip
P = 128                      # partitions = 2 images x 64 channels

F32 = mybir.dt.float32
BF16 = mybir.dt.bfloat16
FP8 = mybir.dt.float8e4
AF = mybir.ActivationFunctionType
ALU = mybir.AluOpType
DR = mybir.MatmulPerfMode.DoubleRow

APITCH = 272                 # act row pitch (bytes %16 for DoubleRow AP steps)
GROUP = 8                    # output rows per PSUM tile (4 banks)

# Param table columns (per-partition f32 scalars)
PK, PB, PS, PB0, PCM, PB1, PSL = 0, 1, 2, 3, 4, 5, 6

# prelu(u) == max(u, slope*u) when 0 <= slope <= 1 (checked at runtime in
# kernel()); one fused scalar_tensor_tensor op instead of tensor_scalar +
# tensor_tensor.  Set False for the general min/mult path.
PRELU_MAX_TRICK = True
USE_LRELU = False            # kept for test.py compat

SIGN_CHUNK = 9               # rows of sign-activation per ACT instruction
STRIP_HS = [32] * 8          # strip heights (sum == H)


def _emit(tc, nc, x_d, w_d, p_d, y_d):
    x3 = x_d.rearrange("p (h w) -> p h w", w=W)
    y3 = y_d.rearrange("p (h w) -> p h w", w=W)

    with ExitStack() as ctx:
        consts = ctx.enter_context(tc.tile_pool(name="consts", bufs=1))
        xpool = ctx.enter_context(tc.tile_pool(name="xpool", bufs=2))
        apool = ctx.enter_context(tc.tile_pool(name="apool", bufs=2))
        ypool = ctx.enter_context(tc.tile_pool(name="ypool", bufs=2))
        vpool = ctx.enter_context(tc.tile_pool(name="vpool", bufs=2))
        pspool = ctx.enter_context(tc.tile_pool(name="pspool", bufs=2,
                                                space="PSUM"))

        # params first on the load ring (sign needs them); weights on the
        # gpsimd ring, which is idle at kernel start -- the first x chunk
        # then starts immediately behind pt instead of behind the weights
        pt = consts.tile([P, 8], F32)
        nc.sync.dma_start(out=pt, in_=p_d)
        # [kw, delta(kh 0/1), m] DoubleRow weights for the kh={0,1} pairs
        wdr = consts.tile([P, 3, 2, 128], FP8)
        nc.gpsimd.dma_start(out=wdr, in_=w_d[:, :768].rearrange(
            "p (k d m) -> p k d m", k=3, d=2))
        # [delta(kw 0/1), m] DoubleRow weights for the kh=2 kw-pair
        w2 = consts.tile([P, 2, 128], FP8)
        nc.gpsimd.dma_start(out=w2, in_=w_d[:, 768:1024].rearrange(
            "p (d m) -> p d m", d=2))
        # [m] plain weights for the lone (kh=2,kw=2) tap
        wn = consts.tile([P, 128], FP8)
        nc.gpsimd.dma_start(out=wn, in_=w_d[:, 1024:1152])

        H0S = [sum(STRIP_HS[:i]) for i in range(len(STRIP_HS))]
        NST = len(STRIP_HS)
        HSMAX = max(STRIP_HS)

        def strip_rows(s):
            h0 = H0S[s]
            row_lo = max(h0 - 1, 0)
            row_hi = min(h0 + STRIP_HS[s] + 1, H)
            return h0, row_lo, row_hi, row_lo - (h0 - 1)

        def load_strip(s):
            """DMA the x strip (rows h0-1 .. h0+hs; tile row a <-> global
            h0-1+a) and memset the act padding."""
            h0, row_lo, row_hi, r0 = strip_rows(s)
            nr = row_hi - row_lo
            xs = xpool.tile([P, HSMAX + 2, W], BF16, name="xs")
            nld = 4 if s == 0 else 2     # strip 0 in quarters: faster start
            bounds = [row_lo + (nr * i) // nld for i in range(nld + 1)]
            for a, b in zip(bounds, bounds[1:]):
                if b > a:
                    nc.sync.dma_start(out=xs[:, a - (h0 - 1):b - (h0 - 1), :],
                                      in_=x3[:, a:b, :])
            # act planes: [plane, row, col]; plane 1 is the +1-column shift
            act = apool.tile([P, 2, HSMAX + 2, APITCH], FP8, name="act")
            nrows = STRIP_HS[s] + 2
            nc.gpsimd.memset(act[:, 0, :nrows, 0:1], 0.0)
            nc.gpsimd.memset(act[:, 0, :nrows, W + 1:W + 2], 0.0)
            if s == 0:
                nc.gpsimd.memset(act[:, :, 0:1, :], 0.0)
            if s == NST - 1:
                nc.gpsimd.memset(act[:, :, nrows - 1:nrows, :], 0.0)
            return xs, act

        def sign_strip(s, xs, act, chunks, skip=0):
            """Binarize x into the zero-padded act plane 0, in row chunks
            (the first small so dependent matmuls unblock quickly), then
            DMA plane 1 = plane 0 shifted left one column."""
            _, row_lo, row_hi, r0 = strip_rows(s)
            c0 = r0 + skip
            for sz in chunks:
                c1 = min(c0 + sz, r0 + (row_hi - row_lo))
                if c1 <= c0:
                    break
                nc.scalar.activation(
                    act[:, 0, c0:c1, 1:W + 1], xs[:, c0:c1, :], AF.Sign,
                    bias=pt[:, PB:PB + 1], scale=pt[:, PK:PK + 1],
                )
                nc.gpsimd.dma_start(out=act[:, 1, c0:c1, 0:W + 1],
                                    in_=act[:, 0, c0:c1, 1:W + 2])
                c0 = c1

        FIRST_CHUNKS = (5,) * 7 + (4,)   # strip 0: progressive chunks
        NEXT_CHUNKS = (5,) + (SIGN_CHUNK,) * 4

        cur = load_strip(0)
        sign_strip(0, *cur, FIRST_CHUNKS)
        nxt = None
        for s in range(NST):
            h0 = H0S[s]
            HS_S = STRIP_HS[s]
            NG = HS_S // GROUP
            xs, act = cur
            ys = ypool.tile([P, HSMAX, W], BF16, name="ys")
            for g in range(NG):
                if g == 1 and s + 1 < NST:
                    nxt = load_strip(s + 1)   # loads overlap this strip
                if g == NG - 1 and s + 1 < NST:
                    # data definitely landed; ACT binarizes the first rows
                    # while the PE finishes this strip
                    sign_strip(s + 1, *nxt, NEXT_CHUNKS[:1])
                ps = pspool.tile([P, GROUP, W], F32, name="ps")
                for j in range(GROUP // 2):
                    rr = GROUP * g + 2 * j   # strip-local first row of pair
                    for kw in range(3):
                        for i in range(2):
                            # kh in {0,1} via DoubleRow: contraction over
                            # (partition, delta), act rows (rr+i)+{0,1}
                            nc.tensor.matmul(
                                ps[:, 2 * j + i, :],
                                lhsT=wdr[:, kw, :, :],
                                rhs=act[:, 0, rr + i:rr + i + 2, kw:kw + W],
                                start=(kw == 0 and i == 0),
                                stop=False,
                                perf_mode=DR,
                            )
                    for i in range(2):
                        # kh=2, kw in {0,1} via DoubleRow across the two
                        # act planes (plane 1 = plane 0 shifted one column)
                        nc.tensor.matmul(
                            ps[:, 2 * j + i, :],
                            lhsT=w2,
                            rhs=act[:, 0:2, rr + i + 2, 0:W],
                            start=False,
                            stop=False,
                            perf_mode=DR,
                        )
                    # lone (kh=2,kw=2) tap: plain matmul over both rows
                    nc.tensor.matmul(
                        ps[:, 2 * j:2 * j + 2, :],
                        lhsT=wn,
                        rhs=act[:, 0, rr + 2:rr + 4, 2:2 + W],
                        start=False,
                        stop=True,
                    )
                # v = ps*scaling + b0: one ACT drain per 8 rows (4 banks)
                v = vpool.tile([P, GROUP, W], BF16, name="v")
                nc.scalar.activation(
                    v, ps, AF.Identity,
                    bias=pt[:, PB0:PB0 + 1], scale=pt[:, PS:PS + 1],
                )
                r0 = GROUP * g
                u = ys[:, r0:r0 + GROUP, :]
                xres = xs[:, r0 + 1:r0 + 1 + GROUP, :]
                if PRELU_MAX_TRICK:
                    nc.vector.scalar_tensor_tensor(
                        u, v, pt[:, PSL:PSL + 1], v, ALU.mult, ALU.max
                    )
                    nc.vector.scalar_tensor_tensor(
                        u, xres, pt[:, PB1:PB1 + 1], u, ALU.add, ALU.add
                    )
                else:
                    # u = v + (slope-1)*min(v, 0); u += x+b1
                    m = vpool.tile([P, GROUP, W], BF16, name="m")
                    nc.vector.tensor_scalar(
                        m, v, 0.0, pt[:, PCM:PCM + 1], ALU.min, ALU.mult
                    )
                    nc.vector.tensor_tensor(u, v, m, ALU.add)
                    nc.vector.scalar_tensor_tensor(
                        u, xres, pt[:, PB1:PB1 + 1], u, ALU.add, ALU.add
                    )
            if s + 1 < NST:
                sign_strip(s + 1, *nxt, NEXT_CHUNKS[1:], skip=NEXT_CHUNKS[0])
            # stores share the gpsimd HWDGE ring with the act-shift copies;
            # they are emitted after the copies so the (PE-gating) copies
            # never queue behind a multi-us store transfer
            for q in range(2):
                r = q * (HS_S // 2)
                r1 = (q + 1) * (HS_S // 2)
                nc.gpsimd.dma_start(out=y3[:, h0 + r:h0 + r1, :],
                                    in_=ys[:, r:r1, :])
            cur = nxt


def build_nc():
    nc = bacc.Bacc("TRN2", target_bir_lowering=False, debug=False,
                   num_devices=NCORES)
    x_d = nc.dram_tensor("xin", [P, H * W], BF16, kind="ExternalInput").ap()
    w_d = nc.dram_tensor("wp", [P, 9 * 128], FP8, kind="ExternalInput").ap()
    p_d = nc.dram_tensor("pp", [P, 8], F32, kind="ExternalInput").ap()
    y_d = nc.dram_tensor("yout", [P, H * W], BF16, kind="ExternalOutput").ap()
    with tile.TileContext(nc) as tc:
        _emit(tc, nc, x_d, w_d, p_d, y_d)
    nc.compile()
    return nc


_NC_CACHE = {}


def _get_nc():
    key = (PRELU_MAX_TRICK,)
    if key not in _NC_CACHE:
        _NC_CACHE[key] = build_nc()
    return _NC_CACHE[key]


def make_inputs(x, rd_k, rd_b, beta, conv_w, pr_bias0, prelu_w, pr_bias1):
    """Host-side prep: per-channel param table, packed sign weights, shards."""
    k = np.asarray(rd_k, np.float32).reshape(C)
    b = np.asarray(rd_b, np.float32).reshape(C)
    s = np.mean(np.abs(np.asarray(conv_w, np.float32)), axis=(1, 2, 3))
    b0 = np.asarray(pr_bias0, np.float32).reshape(C)
    slope = np.asarray(prelu_w, np.float32).reshape(C)
    b1 = np.asarray(pr_bias1, np.float32).reshape(C)
    cm = slope - 1.0
    cols = np.stack([k, b, s, b0, cm, b1, slope, np.zeros(C, np.float32)],
                    axis=1)
    pp = np.concatenate([cols, cols], axis=0).astype(np.float32)  # [128, 8]

    sw = np.sign(np.asarray(conv_w, np.float32)).astype(np.float32)

    def blockdiag(kh, kw):
        S = sw[:, :, kh, kw].T  # [ci, co]
        out = np.zeros((P, P), np.float32)
        out[0:C, 0:C] = S
        out[C:P, C:P] = S
        return out

    wp = np.zeros((P, 9, 128), np.float32)
    for kw in range(3):            # [kw, delta(kh 0/1), m] DoubleRow pairs
        for d in range(2):
            wp[:, kw * 2 + d, :] = blockdiag(d, kw)
    wp[:, 6, :] = blockdiag(2, 0)  # [delta(kw 0/1), m] kh=2 DR pair
    wp[:, 7, :] = blockdiag(2, 1)
    wp[:, 8, :] = blockdiag(2, 2)  # lone (kh=2,kw=2)
    wdt = mybir.dt.np(FP8)
    wp = np.ascontiguousarray(wp.reshape(P, 9 * 128)).astype(wdt)

    x = np.asarray(x, np.float32).astype(ml_dtypes.bfloat16)
    in_maps = []
    for c in range(NCORES):
        xc = np.ascontiguousarray(x[2 * c:2 * c + 2]).reshape(P, H * W)
        in_maps.append({"xin": xc, "wp": wp, "pp": pp})
    return in_maps


def kernel(x, rd_k, rd_b, beta, conv_w, pr_bias0, prelu_w, pr_bias1):
    global PRELU_MAX_TRICK
    slope = np.asarray(prelu_w, np.float32).reshape(C)
    if not np.all((slope >= 0.0) & (slope <= 1.0)):
        PRELU_MAX_TRICK = False   # max-identity only valid for slope in [0,1]
    in_maps = make_inputs(x, rd_k, rd_b, beta, conv_w, pr_bias0, prelu_w,
                          pr_bias1)
    nc = _get_nc()
    res = run_bass_kernel_spmd(nc, in_maps, core_ids=list(range(NCORES)))
    y = np.empty((B, C, H, W), np.float32)
    for c in range(NCORES):
        y[2 * c:2 * c + 2] = np.asarray(
            res.results[c]["yout"], dtype=np.float32).reshape(2, C, H, W)
    return y
